# revision 32
# baseline (speedup 1.0000x reference)
"""Trainium2 Bass kernel for nn_LongThinNet — v9: uniform 12-lane tiles.

Per core 131072 rows = 132 lanes of 1024 rows. 11 uniform tiles
[128, 1024]: 12 lanes x 1024 batch cols (last tile 8 lanes, x zero-padded);
lane ll sits at partitions 32*(ll//3) + 10*(ll%3) .. +10; partition row 30
carries a constant 1.0 so every layer's bias rides in the matmul (row 30 of
each weight block = bias, [30,30] = 1.0 keeps it alive). No special-case
"C" region at all — one weight block serves every tile.

4 psum chains of TWO banks each; chain i runs tiles u = i mod 4 through all
16 layers, with the L15 [128, 1024] output psum drawn from the chain's own
pool (no separate output pool). Emission is layer-major across all 11
tiles, so the chains stagger naturally and each bank round-robins its ~3
tiles every layer.

Elementwise: ACT native Prelu (1 instr) vs DVE legal 2-pass (u = 0.5*psum,
single PSUM read; then all-SBUF max(2u, u)); a greedy balancer levels the
two engines. l=13 act stays on ACT (bf16 output so the h-stationary L15
matmuls run at 1 cycle/row instead of fp32r's small-free-dim 4x penalty).
"""

import sys

sys.path.insert(0, "/opt/trn_rl_repo")

from contextlib import ExitStack

import ml_dtypes
import numpy as np

import concourse.bass as bass
import concourse.mybir as mybir
import concourse.tile as tile
from concourse.bass_utils import run_bass_kernel_spmd

F32 = mybir.dt.float32
F32R = mybir.dt.float32r
BF16 = mybir.dt.bfloat16
AF = mybir.ActivationFunctionType
ALU = mybir.AluOpType

NPBF16 = ml_dtypes.bfloat16

NCORES = 8
BC = 131072
IN, HID = 40, 10
NMID = 14
NT = 11              # tiles per core
NCH = 4              # concurrent psum chains
PAD = NT * 12 * 1024  # 135168 padded rows per core

BANDS = [(b, i) for b in range(4) for i in range(3)]  # lane ll = 3b+i


def _skip(name):
    return name in ("InstEventSemaphore", "InstAllEngineBarrier")


def _split_multi_waits(nc):
    """walrus codegen allows <=1 semaphore wait per instruction; hoist extras
    onto standalone InstEventSemaphore instructions inserted just before."""
    n_new = 0
    for f in nc.m.functions:
        for bb in f.blocks:
            out, changed = [], False
            for inst in bb.instructions:
                si = inst.sync_info
                if si is not None and len(si.on_wait) > 1 and not _skip(type(inst).__name__):
                    waits = list(si.on_wait)
                    for w in waits[:-1]:
                        n_new += 1
                        out.append(
                            mybir.InstEventSemaphore(
                                name=f"EVW-{n_new}-{inst.name}",
                                engine=inst.engine,
                                sync_info=mybir.SyncInfo(on_wait=[w], on_update=[]),
                            )
                        )
                    inst.sync_info = mybir.SyncInfo(
                        on_wait=[waits[-1]], on_update=list(si.on_update)
                    )
                    changed = True
                out.append(inst)
            if changed:
                try:
                    bb.instructions = out
                except Exception:
                    lst = bb.instructions
                    lst.clear()
                    lst.extend(out)
    return n_new


def _pack_weights(W_in, b_in, W_mid, b_mid, W_out, b_out):
    # L0: x chunk k holds lanes 3k+gamma at partitions 40*gamma; wl0a block k
    # maps them to out partitions 32k + 10*gamma.
    wl0a = np.zeros((120, 4 * 128), np.float32)
    for k in range(4):
        for g in range(3):
            wl0a[40 * g:40 * g + 40,
                 128 * k + 32 * k + 10 * g:128 * k + 32 * k + 10 * g + 10] = W_in.T

    wmid = np.zeros((128, NMID * 128), np.float32)
    for l in range(NMID):
        for b, i in BANDS:
            q = 32 * b + 10 * i
            wmid[q:q + 10, 128 * l + q:128 * l + q + 10] = W_mid[l].T
            wmid[30, 128 * l + q:128 * l + q + 10] = b_mid[l]
        wmid[30, 128 * l + 30] = 1.0

    wl15 = np.zeros((128, 120), np.float32)
    for b, i in BANDS:
        q = 32 * b + 10 * i
        jl = 3 * b + i
        wl15[q:q + 10, 10 * jl:10 * jl + 10] = W_out.T
        wl15[30, 10 * jl:10 * jl + 10] = b_out

    wbias = np.zeros((128, 1), np.float32)
    for b, i in BANDS:
        q = 32 * b + 10 * i
        wbias[q:q + 10, 0] = b_in
    wbias[30, 0] = 1.0

    return {"wl0a": wl0a.astype(NPBF16), "wmid": wmid,
            "wl15": wl15.astype(NPBF16), "wbias": wbias}


def _pack_x_core(xc):
    """[131072, 40] -> bf16 [NT, 120, 4096]: tile u, partition 40*gamma+f,
    free 2048h+512k+c holds x[1024*(12u + 3k+gamma) + 512h + c, f] (zero
    pad past BC); halves are contiguous so L0 can start after half a tile."""
    pad = np.zeros((PAD, IN), np.float32)
    pad[:BC] = xc
    xr = pad.reshape(NT, 4, 3, 2, 512, IN)       # [u, k, gamma, h, c, f]
    xp = xr.transpose(0, 2, 5, 3, 1, 4)          # [u, gamma, f, h, k, c]
    return np.ascontiguousarray(xp.reshape(NT, 120, 4096).astype(NPBF16))


def unpack_out(res_out):
    """res_out: [NCORES, NT, 128, 960] -> [BATCH, HID].
    out[u, p, 480h + 120gg + 10*ll + o] = y[1024*(12u+ll) + 512h + 128gg + p]."""
    outs = []
    for c in range(NCORES):
        o5 = res_out[c].reshape(NT, 128, 2, 4, 12, HID)  # [u,p,h,gg,ll,o]
        y = o5.transpose(0, 4, 2, 3, 1, 5).reshape(NT * 12, 1024, HID)
        outs.append(y.reshape(PAD, HID)[:BC])
    return np.ascontiguousarray(np.concatenate(outs, axis=0))


def _build_nc(reps=1):
    nc = bass.Bass("TRN2", target_bir_lowering=False, debug=False)

    x_d = nc.dram_tensor("x", [NT, 120, 4096], BF16, kind="ExternalInput").ap()
    wl0a_d = nc.dram_tensor("wl0a", [120, 512], BF16, kind="ExternalInput").ap()
    wmid_d = nc.dram_tensor("wmid", [128, NMID * 128], F32R, kind="ExternalInput").ap()
    wl15_d = nc.dram_tensor("wl15", [128, 120], BF16, kind="ExternalInput").ap()
    wbias_d = nc.dram_tensor("wbias", [128, 1], F32, kind="ExternalInput").ap()
    out_d = nc.dram_tensor("out", [NT, 128, 960], F32, kind="ExternalOutput").ap()

    with tile.TileContext(nc) as tc, ExitStack() as ctx:
        sc = ctx.enter_context(tc.tile_pool(name="sc", bufs=1))
        sx = ctx.enter_context(tc.tile_pool(name="sx", bufs=8))
        shp = [ctx.enter_context(tc.tile_pool(name=f"sh{i}", bufs=4))
               for i in range(NCH)]
        su = ctx.enter_context(tc.tile_pool(name="su", bufs=4))
        sout = ctx.enter_context(tc.tile_pool(name="sout", bufs=4))
        pch = [ctx.enter_context(tc.tile_pool(name=f"pc{i}", bufs=1, space="PSUM"))
               for i in range(NCH)]

        consts = {}

        def _load_consts():
            for name, dram, shape, dt in [
                ("wl0a", wl0a_d, [120, 512], BF16),
                ("wbias", wbias_d, [128, 1], F32),
                ("wmid", wmid_d, [128, NMID * 128], F32R),
                ("wl15", wl15_d, [128, 120], BF16),
            ]:
                t = sc.tile(shape, dt, name=f"c_{name}", tag=name)
                nc.sync.dma_start(t[:], dram)
                consts[name] = t

        # Greedy two-engine balancer (running busy-time estimates, us).
        eng = {"A": 0.0, "D": 0.0}
        ACT_COST = {1024: 1.038, 480: 0.585}
        DVE_COST = {1024: 2.50, 480: 0.625}

        def emit_act(dst, psum, force_act=False, bias=False):
            cost_a = max(eng["A"] + ACT_COST[1024], eng["D"])
            cost_d = max(eng["A"], eng["D"] + DVE_COST[1024])
            if force_act or cost_a <= cost_d:
                eng["A"] += ACT_COST[1024]
                b = consts["wbias"][:, 0:1] if bias else 0.0
                nc.scalar.activation(dst, psum, AF.Prelu,
                                     bias=b, scale=1.0, alpha=0.5)
            else:
                eng["D"] += DVE_COST[1024]
                u = su.tile([128, 1024], F32, name="u", tag="u")
                if bias:
                    nc.vector.tensor_scalar(u[:], psum, consts["wbias"][:, 0:1],
                                            0.5, ALU.add, ALU.mult)
                else:
                    nc.vector.tensor_scalar_mul(u[:], psum, 0.5)
                nc.vector.scalar_tensor_tensor(dst, u[:], 2.0, u[:],
                                               ALU.mult, ALU.max)

        def emit_copy(dst, psum):
            if eng["A"] + ACT_COST[480] <= eng["D"] + DVE_COST[480]:
                eng["A"] += ACT_COST[480]
                nc.scalar.activation(dst, psum, AF.Copy)
            else:
                eng["D"] += DVE_COST[480]
                nc.vector.tensor_copy(dst, psum)

        loop_ctx = tc.For_i(0, reps, 1) if reps > 1 else None
        if loop_ctx is not None:
            ctx.enter_context(loop_ctx)

        _load_consts()
        x_t = []
        for u in range(NT):
            x_t.append(sx.tile([120, 4096], BF16, name=f"x{u}", tag="x"))
        for u in range(min(NCH, NT)):
            nc.sync.dma_start(x_t[u][:, 0:2048], x_d[u][:, 0:2048])
        for u in range(min(NCH, NT)):
            nc.sync.dma_start(x_t[u][:, 2048:4096], x_d[u][:, 2048:4096])
        for u in range(NCH, NT):
            nc.sync.dma_start(x_t[u][:, 0:2048], x_d[u][:, 0:2048])
            nc.sync.dma_start(x_t[u][:, 2048:4096], x_d[u][:, 2048:4096])

        s_h = {}
        # L0: per tile 4 accumulating matmuls per 512-col half; act seeds
        # the constant row via the bias vector.
        for u in range(NT):
            ch = u % NCH
            p = pch[ch].tile([128, 1024], F32, name=f"p{ch}", tag="p")
            for half in range(2):
                for k in range(4):
                    nc.tensor.matmul(
                        p[:, 512 * half:512 * half + 512],
                        consts["wl0a"][:, 128 * k:128 * k + 128],
                        x_t[u][:, 2048 * half + 512 * k:2048 * half + 512 * k + 512],
                        start=(k == 0), stop=(k == 3))
            s = shp[ch].tile([128, 1024], F32R, name=f"s{ch}", tag="h")
            emit_act(s[:], p[:], bias=True)
            s_h[u] = s

        # 14 middle layers, layer-major across all tiles: each chain's bank
        # round-robins its ~3 tiles, so the chains stay staggered.
        for l in range(NMID):
            sdt = BF16 if l == NMID - 1 else F32R
            wm = consts["wmid"][:, 128 * l:128 * l + 128]
            np_h = {}
            for u in range(NT):
                ch = u % NCH
                np_h[u] = pch[ch].tile([128, 1024], F32, name=f"p{ch}", tag="p")
                nc.tensor.matmul(np_h[u][:, 0:512], wm, s_h[u][:, 0:512],
                                 start=True, stop=True)
                nc.tensor.matmul(np_h[u][:, 512:1024], wm, s_h[u][:, 512:1024],
                                 start=True, stop=True)
            for u in range(NT):
                ch = u % NCH
                ns = shp[ch].tile([128, 1024], sdt, name=f"s{ch}", tag="h")
                emit_act(ns[:], np_h[u][:])
                s_h[u] = ns

        # L15: h stationary, 8 groups of 128 batch cols; outputs packed at
        # 512*(g//4) + 120*(g%4) so no matmul crosses a psum bank. The
        # output psum is the chain's own next pool tile.
        for u in range(NT):
            ch = u % NCH
            po = pch[ch].tile([128, 1024], F32, name=f"p{ch}", tag="p")
            for g in range(8):
                off = 512 * (g // 4) + 120 * (g % 4)
                nc.tensor.matmul(po[:, off:off + 120],
                                 s_h[u][:, 128 * g:128 * g + 128],
                                 consts["wl15"][:],
                                 start=True, stop=True)
            s_o = sout.tile([128, 960], F32, name="so", tag="out")
            emit_copy(s_o[:, 0:480], po[:, 0:480])
            emit_copy(s_o[:, 480:960], po[:, 512:992])
            nc.sync.dma_start(out_d[u], s_o[:])

    _split_multi_waits(nc)
    return nc


_NC_CACHE = {}


def build_in_maps(np_inputs):
    x = np.asarray(np_inputs["x"], np.float32)
    consts = _pack_weights(
        np.asarray(np_inputs["W_in"], np.float32),
        np.asarray(np_inputs["b_in"], np.float32),
        np.asarray(np_inputs["W_mid"], np.float32),
        np.asarray(np_inputs["b_mid"], np.float32),
        np.asarray(np_inputs["W_out"], np.float32),
        np.asarray(np_inputs["b_out"], np.float32),
    )
    in_maps = []
    for c in range(NCORES):
        xc = _pack_x_core(x[c * BC:(c + 1) * BC])
        in_maps.append({"x": xc, **consts})
    return in_maps


def kernel(x, W_in, b_in, W_mid, b_mid, W_out, b_out):
    if "nc" not in _NC_CACHE:
        _NC_CACHE["nc"] = _build_nc()
    nc = _NC_CACHE["nc"]

    in_maps = build_in_maps(dict(x=x, W_in=W_in, b_in=b_in, W_mid=W_mid,
                                 b_mid=b_mid, W_out=W_out, b_out=b_out))

    res = run_bass_kernel_spmd(nc, in_maps, list(range(NCORES)))

    return unpack_out(np.stack([res.results[c]["out"] for c in range(NCORES)]))


def emulate_core(xc, consts):
    """Numpy emulation of the per-core kernel semantics."""
    xp = _pack_x_core(xc).astype(np.float32)     # [NT, 120, 4096]
    wl0a = consts["wl0a"].astype(np.float32)
    wmid = consts["wmid"]
    wl15 = consts["wl15"].astype(np.float32)
    wbias = consts["wbias"][:, 0]
    out = np.zeros((NT, 128, 960), np.float32)
    for u in range(NT):
        p = np.zeros((128, 1024), np.float32)
        for half in range(2):
            for k in range(4):
                sl = slice(2048 * half + 512 * k, 2048 * half + 512 * k + 512)
                p[:, 512 * half:512 * half + 512] += (
                    wl0a[:, 128 * k:128 * k + 128].T @ xp[u][:, sl])
        z = p + wbias[:, None]
        s = np.maximum(z, 0.5 * z)
        for l in range(NMID):
            z = wmid[:, 128 * l:128 * l + 128].T @ s
            s = np.maximum(z, 0.5 * z)
        po = np.zeros((128, 1024), np.float32)
        for g in range(8):
            off = 512 * (g // 4) + 120 * (g % 4)
            po[:, off:off + 120] = s[:, 128 * g:128 * g + 128].T @ wl15
        out[u][:, 0:480] = po[:, 0:480]
        out[u][:, 480:960] = po[:, 512:992]
    return out


if __name__ == "__main__":
    rng = np.random.default_rng(0)
    B = BC
    x = rng.standard_normal((B, IN), dtype=np.float32)
    W_in = rng.standard_normal((HID, IN), dtype=np.float32) * 0.1
    b_in = rng.standard_normal(HID).astype(np.float32) * 0.1
    W_mid = rng.standard_normal((NMID, HID, HID), dtype=np.float32) * 0.1
    b_mid = rng.standard_normal((NMID, HID)).astype(np.float32) * 0.1
    W_out = rng.standard_normal((HID, HID), dtype=np.float32) * 0.1
    b_out = rng.standard_normal(HID).astype(np.float32) * 0.1

    def act(v):
        return np.maximum(v, 0.5 * v)

    h = act(x @ W_in.T + b_in)
    for l in range(NMID):
        h = act(h @ W_mid[l].T + b_mid[l])
    ref = h @ W_out.T + b_out

    consts = _pack_weights(W_in, b_in, W_mid, b_mid, W_out, b_out)
    res = emulate_core(x, consts)
    o5 = res.reshape(NT, 128, 2, 4, 12, HID)
    got = o5.transpose(0, 4, 2, 3, 1, 5).reshape(PAD, HID)[:B]
    err = np.abs(got - ref).max() / np.abs(ref).max()
    print("emulation rel err:", err)


# revision 33
# speedup vs baseline: 1.1099x; 1.1099x over previous
"""Trainium2 Bass kernel for nn_LongThinNet — v9: uniform 12-lane tiles.

Per core 131072 rows = 132 lanes of 1024 rows. 11 uniform tiles
[128, 1024]: 12 lanes x 1024 batch cols (last tile 8 lanes, x zero-padded);
lane ll sits at partitions 32*(ll//3) + 10*(ll%3) .. +10; partition row 30
carries a constant 1.0 so every layer's bias rides in the matmul (row 30 of
each weight block = bias, [30,30] = 1.0 keeps it alive). No special-case
"C" region at all — one weight block serves every tile.

4 psum chains of TWO banks each; chain i runs tiles u = i mod 4 through all
16 layers, with the L15 [128, 1024] output psum drawn from the chain's own
pool (no separate output pool). Emission is layer-major across all 11
tiles, so the chains stagger naturally and each bank round-robins its ~3
tiles every layer.

Elementwise: ACT native Prelu (1 instr) vs DVE legal 2-pass (u = 0.5*psum,
single PSUM read; then all-SBUF max(2u, u)); a greedy balancer levels the
two engines. l=13 act stays on ACT (bf16 output so the h-stationary L15
matmuls run at 1 cycle/row instead of fp32r's small-free-dim 4x penalty).
"""

import sys

sys.path.insert(0, "/opt/trn_rl_repo")

from contextlib import ExitStack

import ml_dtypes
import numpy as np

import concourse.bass as bass
import concourse.mybir as mybir
import concourse.tile as tile
from concourse.bass_utils import run_bass_kernel_spmd

F32 = mybir.dt.float32
F32R = mybir.dt.float32r
BF16 = mybir.dt.bfloat16
AF = mybir.ActivationFunctionType
ALU = mybir.AluOpType

NPBF16 = ml_dtypes.bfloat16

NCORES = 8
BC = 131072
IN, HID = 40, 10
NMID = 14
NT = 11              # tiles per core
NCH = 4              # concurrent psum chains
PAD = NT * 12 * 1024  # 135168 padded rows per core

BANDS = [(b, i) for b in range(4) for i in range(3)]  # lane ll = 3b+i


def _skip(name):
    return name in ("InstEventSemaphore", "InstAllEngineBarrier")


def _split_multi_waits(nc):
    """walrus codegen allows <=1 semaphore wait per instruction; hoist extras
    onto standalone InstEventSemaphore instructions inserted just before."""
    n_new = 0
    for f in nc.m.functions:
        for bb in f.blocks:
            out, changed = [], False
            for inst in bb.instructions:
                si = inst.sync_info
                if si is not None and len(si.on_wait) > 1 and not _skip(type(inst).__name__):
                    waits = list(si.on_wait)
                    for w in waits[:-1]:
                        n_new += 1
                        out.append(
                            mybir.InstEventSemaphore(
                                name=f"EVW-{n_new}-{inst.name}",
                                engine=inst.engine,
                                sync_info=mybir.SyncInfo(on_wait=[w], on_update=[]),
                            )
                        )
                    inst.sync_info = mybir.SyncInfo(
                        on_wait=[waits[-1]], on_update=list(si.on_update)
                    )
                    changed = True
                out.append(inst)
            if changed:
                try:
                    bb.instructions = out
                except Exception:
                    lst = bb.instructions
                    lst.clear()
                    lst.extend(out)
    return n_new


def _pack_weights(W_in, b_in, W_mid, b_mid, W_out, b_out):
    # L0: x chunk k holds lanes 3k+gamma at partitions 40*gamma; wl0a block k
    # maps them to out partitions 32k + 10*gamma.
    wl0a = np.zeros((120, 4 * 128), np.float32)
    for k in range(4):
        for g in range(3):
            wl0a[40 * g:40 * g + 40,
                 128 * k + 32 * k + 10 * g:128 * k + 32 * k + 10 * g + 10] = W_in.T

    wmid = np.zeros((128, NMID * 128), np.float32)
    for l in range(NMID):
        for b, i in BANDS:
            q = 32 * b + 10 * i
            wmid[q:q + 10, 128 * l + q:128 * l + q + 10] = W_mid[l].T
            wmid[30, 128 * l + q:128 * l + q + 10] = b_mid[l]
        wmid[30, 128 * l + 30] = 1.0

    wl15 = np.zeros((128, 120), np.float32)
    for b, i in BANDS:
        q = 32 * b + 10 * i
        jl = 3 * b + i
        wl15[q:q + 10, 10 * jl:10 * jl + 10] = W_out.T
        wl15[30, 10 * jl:10 * jl + 10] = b_out

    wbias = np.zeros((128, 1), np.float32)
    for b, i in BANDS:
        q = 32 * b + 10 * i
        wbias[q:q + 10, 0] = b_in
    wbias[30, 0] = 1.0

    return {"wl0a": wl0a.astype(NPBF16), "wmid": wmid,
            "wl15": wl15.astype(NPBF16), "wbias": wbias}


def _pack_x_core(xc):
    """[131072, 40] -> bf16 [NT, 120, 4096]: tile u, partition 40*gamma+f,
    free 2048h+512k+c holds x[1024*(12u + 3k+gamma) + 512h + c, f] (zero
    pad past BC); halves are contiguous so L0 can start after half a tile."""
    pad = np.zeros((PAD, IN), np.float32)
    pad[:BC] = xc
    xr = pad.reshape(NT, 4, 3, 2, 512, IN)       # [u, k, gamma, h, c, f]
    xp = xr.transpose(0, 2, 5, 3, 1, 4)          # [u, gamma, f, h, k, c]
    return np.ascontiguousarray(xp.reshape(NT, 120, 4096).astype(NPBF16))


def unpack_out(res_out):
    """res_out: [NCORES, NT, 128, 960] -> [BATCH, HID].
    out[u, p, 480h + 120gg + 10*ll + o] = y[1024*(12u+ll) + 512h + 128gg + p]."""
    outs = []
    for c in range(NCORES):
        o5 = res_out[c].reshape(NT, 128, 2, 4, 12, HID)  # [u,p,h,gg,ll,o]
        y = o5.transpose(0, 4, 2, 3, 1, 5).reshape(NT * 12, 1024, HID)
        outs.append(y.reshape(PAD, HID)[:BC])
    return np.ascontiguousarray(np.concatenate(outs, axis=0))


def _build_nc(reps=1):
    nc = bass.Bass("TRN2", target_bir_lowering=False, debug=False)

    x_d = nc.dram_tensor("x", [NT, 120, 4096], BF16, kind="ExternalInput").ap()
    wl0a_d = nc.dram_tensor("wl0a", [120, 512], BF16, kind="ExternalInput").ap()
    wmid_d = nc.dram_tensor("wmid", [128, NMID * 128], F32R, kind="ExternalInput").ap()
    wl15_d = nc.dram_tensor("wl15", [128, 120], BF16, kind="ExternalInput").ap()
    wbias_d = nc.dram_tensor("wbias", [128, 1], F32, kind="ExternalInput").ap()
    out_d = nc.dram_tensor("out", [NT, 128, 960], F32, kind="ExternalOutput").ap()

    with tile.TileContext(nc) as tc, ExitStack() as ctx:
        sc = ctx.enter_context(tc.tile_pool(name="sc", bufs=1))
        sx = ctx.enter_context(tc.tile_pool(name="sx", bufs=8))
        shp = [ctx.enter_context(tc.tile_pool(name=f"sh{i}", bufs=4))
               for i in range(NCH)]
        su = ctx.enter_context(tc.tile_pool(name="su", bufs=4))
        sout = ctx.enter_context(tc.tile_pool(name="sout", bufs=4))
        pch = [ctx.enter_context(tc.tile_pool(name=f"pc{i}", bufs=1, space="PSUM"))
               for i in range(NCH)]

        consts = {}

        def _load_consts():
            for name, dram, shape, dt in [
                ("wl0a", wl0a_d, [120, 512], BF16),
                ("wbias", wbias_d, [128, 1], F32),
                ("wmid", wmid_d, [128, NMID * 128], F32R),
                ("wl15", wl15_d, [128, 120], BF16),
            ]:
                t = sc.tile(shape, dt, name=f"c_{name}", tag=name)
                nc.sync.dma_start(t[:], dram)
                consts[name] = t

        # Greedy two-engine balancer (running busy-time estimates, us).
        eng = {"A": 0.0, "D": 0.0}
        ACT_COST = {1024: 1.038, 480: 0.585}
        DVE_COST = {1024: 2.35, 480: 0.625}

        def emit_act(dst, psum, force_act=False, bias=False):
            cost_a = max(eng["A"] + ACT_COST[1024], eng["D"])
            cost_d = max(eng["A"], eng["D"] + DVE_COST[1024])
            if force_act or cost_a <= cost_d:
                eng["A"] += ACT_COST[1024]
                b = consts["wbias"][:, 0:1] if bias else 0.0
                nc.scalar.activation(dst, psum, AF.Prelu,
                                     bias=b, scale=1.0, alpha=0.5)
            else:
                eng["D"] += DVE_COST[1024]
                u = su.tile([128, 1024], F32, name="u", tag="u")
                if bias:
                    nc.vector.tensor_scalar(u[:], psum, consts["wbias"][:, 0:1],
                                            0.5, ALU.add, ALU.mult)
                else:
                    nc.vector.tensor_scalar_mul(u[:], psum, 0.5)
                nc.vector.scalar_tensor_tensor(dst, u[:], 2.0, u[:],
                                               ALU.mult, ALU.max)

        def emit_copy(dst, psum):
            if eng["A"] + ACT_COST[480] <= eng["D"] + DVE_COST[480]:
                eng["A"] += ACT_COST[480]
                nc.scalar.activation(dst, psum, AF.Copy)
            else:
                eng["D"] += DVE_COST[480]
                nc.vector.tensor_copy(dst, psum)

        loop_ctx = tc.For_i(0, reps, 1) if reps > 1 else None
        if loop_ctx is not None:
            ctx.enter_context(loop_ctx)

        _load_consts()
        x_t = []
        for u in range(NT):
            x_t.append(sx.tile([120, 4096], BF16, name=f"x{u}", tag="x"))
        for u in range(min(NCH, NT)):
            nc.sync.dma_start(x_t[u][:, 0:2048], x_d[u][:, 0:2048])
        for u in range(min(NCH, NT)):
            nc.sync.dma_start(x_t[u][:, 2048:4096], x_d[u][:, 2048:4096])
        for u in range(NCH, NT):
            nc.sync.dma_start(x_t[u][:, 0:2048], x_d[u][:, 0:2048])
            nc.sync.dma_start(x_t[u][:, 2048:4096], x_d[u][:, 2048:4096])

        s_h = {}
        # L0: per tile 4 accumulating matmuls per 512-col half; act seeds
        # the constant row via the bias vector.
        for u in range(NT):
            ch = u % NCH
            p = pch[ch].tile([128, 1024], F32, name=f"p{ch}", tag="p")
            for half in range(2):
                for k in range(4):
                    nc.tensor.matmul(
                        p[:, 512 * half:512 * half + 512],
                        consts["wl0a"][:, 128 * k:128 * k + 128],
                        x_t[u][:, 2048 * half + 512 * k:2048 * half + 512 * k + 512],
                        start=(k == 0), stop=(k == 3))
            s = shp[ch].tile([128, 1024], F32R, name=f"s{ch}", tag="h")
            emit_act(s[:], p[:], bias=True)
            s_h[u] = s

        # 14 middle layers, layer-major across all tiles: each chain's bank
        # round-robins its ~3 tiles, so the chains stay staggered.
        for l in range(NMID):
            sdt = BF16 if l == NMID - 1 else F32R
            wm = consts["wmid"][:, 128 * l:128 * l + 128]
            np_h = {}
            for u in range(NT):
                ch = u % NCH
                np_h[u] = pch[ch].tile([128, 1024], F32, name=f"p{ch}", tag="p")
                nc.tensor.matmul(np_h[u][:, 0:512], wm, s_h[u][:, 0:512],
                                 start=True, stop=True)
                nc.tensor.matmul(np_h[u][:, 512:1024], wm, s_h[u][:, 512:1024],
                                 start=True, stop=True)
            for u in range(NT):
                ch = u % NCH
                ns = shp[ch].tile([128, 1024], sdt, name=f"s{ch}", tag="h")
                emit_act(ns[:], np_h[u][:])
                s_h[u] = ns

        # L15: h stationary, 8 groups of 128 batch cols; outputs packed at
        # 512*(g//4) + 120*(g%4) so no matmul crosses a psum bank. The
        # output psum is the chain's own next pool tile.
        for u in range(NT):
            ch = u % NCH
            po = pch[ch].tile([128, 1024], F32, name=f"p{ch}", tag="p")
            for g in range(8):
                off = 512 * (g // 4) + 120 * (g % 4)
                nc.tensor.matmul(po[:, off:off + 120],
                                 s_h[u][:, 128 * g:128 * g + 128],
                                 consts["wl15"][:],
                                 start=True, stop=True)
            s_o = sout.tile([128, 960], F32, name="so", tag="out")
            emit_copy(s_o[:, 0:480], po[:, 0:480])
            emit_copy(s_o[:, 480:960], po[:, 512:992])
            nc.sync.dma_start(out_d[u], s_o[:])

    _split_multi_waits(nc)
    return nc


_NC_CACHE = {}


def build_in_maps(np_inputs):
    x = np.asarray(np_inputs["x"], np.float32)
    consts = _pack_weights(
        np.asarray(np_inputs["W_in"], np.float32),
        np.asarray(np_inputs["b_in"], np.float32),
        np.asarray(np_inputs["W_mid"], np.float32),
        np.asarray(np_inputs["b_mid"], np.float32),
        np.asarray(np_inputs["W_out"], np.float32),
        np.asarray(np_inputs["b_out"], np.float32),
    )
    in_maps = []
    for c in range(NCORES):
        xc = _pack_x_core(x[c * BC:(c + 1) * BC])
        in_maps.append({"x": xc, **consts})
    return in_maps


def kernel(x, W_in, b_in, W_mid, b_mid, W_out, b_out):
    if "nc" not in _NC_CACHE:
        _NC_CACHE["nc"] = _build_nc()
    nc = _NC_CACHE["nc"]

    in_maps = build_in_maps(dict(x=x, W_in=W_in, b_in=b_in, W_mid=W_mid,
                                 b_mid=b_mid, W_out=W_out, b_out=b_out))

    res = run_bass_kernel_spmd(nc, in_maps, list(range(NCORES)))

    return unpack_out(np.stack([res.results[c]["out"] for c in range(NCORES)]))


def emulate_core(xc, consts):
    """Numpy emulation of the per-core kernel semantics."""
    xp = _pack_x_core(xc).astype(np.float32)     # [NT, 120, 4096]
    wl0a = consts["wl0a"].astype(np.float32)
    wmid = consts["wmid"]
    wl15 = consts["wl15"].astype(np.float32)
    wbias = consts["wbias"][:, 0]
    out = np.zeros((NT, 128, 960), np.float32)
    for u in range(NT):
        p = np.zeros((128, 1024), np.float32)
        for half in range(2):
            for k in range(4):
                sl = slice(2048 * half + 512 * k, 2048 * half + 512 * k + 512)
                p[:, 512 * half:512 * half + 512] += (
                    wl0a[:, 128 * k:128 * k + 128].T @ xp[u][:, sl])
        z = p + wbias[:, None]
        s = np.maximum(z, 0.5 * z)
        for l in range(NMID):
            z = wmid[:, 128 * l:128 * l + 128].T @ s
            s = np.maximum(z, 0.5 * z)
        po = np.zeros((128, 1024), np.float32)
        for g in range(8):
            off = 512 * (g // 4) + 120 * (g % 4)
            po[:, off:off + 120] = s[:, 128 * g:128 * g + 128].T @ wl15
        out[u][:, 0:480] = po[:, 0:480]
        out[u][:, 480:960] = po[:, 512:992]
    return out


if __name__ == "__main__":
    rng = np.random.default_rng(0)
    B = BC
    x = rng.standard_normal((B, IN), dtype=np.float32)
    W_in = rng.standard_normal((HID, IN), dtype=np.float32) * 0.1
    b_in = rng.standard_normal(HID).astype(np.float32) * 0.1
    W_mid = rng.standard_normal((NMID, HID, HID), dtype=np.float32) * 0.1
    b_mid = rng.standard_normal((NMID, HID)).astype(np.float32) * 0.1
    W_out = rng.standard_normal((HID, HID), dtype=np.float32) * 0.1
    b_out = rng.standard_normal(HID).astype(np.float32) * 0.1

    def act(v):
        return np.maximum(v, 0.5 * v)

    h = act(x @ W_in.T + b_in)
    for l in range(NMID):
        h = act(h @ W_mid[l].T + b_mid[l])
    ref = h @ W_out.T + b_out

    consts = _pack_weights(W_in, b_in, W_mid, b_mid, W_out, b_out)
    res = emulate_core(x, consts)
    o5 = res.reshape(NT, 128, 2, 4, 12, HID)
    got = o5.transpose(0, 4, 2, 3, 1, 5).reshape(PAD, HID)[:B]
    err = np.abs(got - ref).max() / np.abs(ref).max()
    print("emulation rel err:", err)
